# revision 36
# baseline (speedup 1.0000x reference)
"""Trainium2 Bass kernel for nn_AdaptiveMobiusLayer.

Strategy (pure data parallel over tokens, 8 NeuronCores):
  - Flatten x [4, 4096, 1024] -> [16384, 1024] tokens; core c takes 2048
    consecutive tokens (= batch b = c//2, seq half c%2).
  - Host transposes each shard to [1024 feats, 2048 tokens] and ships it
    twice: bf16 (the in-SBUF `out` carrier) and fp8 (cycle-0 matmul moving
    operand), so the device does no cycle-0 conversions.
  - Feature-major tiles: every matmul keeps features on partitions (weights
    are natural [K, M] lhsT stationary operands, activations moving).
  - The seq-mean for the global context needs the partner core's half of the
    batch row: the partner's fp8 shard is replicated to each core and summed
    locally during cycle 0 — replacing a pairwise AllReduce whose end-to-end
    latency (~19us) dominated the cycle-0 critical path.
  - MLP matmuls run in fp8 DoubleRow (fp32 accumulation in PSUM); `out` is
    carried in bf16 so the twist-update tensor_tensor ops hit the DVE 2x_1p
    fast path.
  - All sigmoids are computed as 0.5 + 0.5*tanh(z/2); the coupling affine
    c0' + cmul'*tanh rides the partition-broadcast as two accumulating K=1
    matmuls, so ACT only ever uses the gelu table (gelu/tanh share it -> no
    ACT_TABLE_LOAD thrash):
      coupling = (0.1 + 0.7*ar*gf_t) + (0.3*ar)*tanh((z4+b4)/2),
      gf_t = tanh((gz+gb3)/2).
  - Emission order software-pipelines the in-order engine queues: cycle-0
    layers staggered across chunks (DMA delivery pacing), gc-net stages and
    twist updates hooked between MLP layers, updates lag-2 across the
    cycle-0/1 boundary catching back to lag-1 so the tail stays one update
    deep.  All DRAM tensors are host-packed into exact SBUF tile layouts so
    every DMA row is partition-contiguous (dma_start issue ~0.6us sequencer
    time each; ~36-70 GB/s per channel, ~2 concurrent per HWDGE ring).
"""

import sys

sys.path.insert(0, "/opt/trn_rl_repo")

import numpy as np

B, S, DIM = 4, 4096, 1024
NCORES = 8
TOK = B * S // NCORES  # 2048 tokens per core
CHUNK = 512
NCHUNK = TOK // CHUNK  # 4
NUM_CYCLES = 3
BASE_COUPLING = 0.1

# feature-quarter twist:  out_new[t] = out[t] + sign[t] * c * out[(t+4) % 8]
# tiles 0..7 are 128-feature slabs; quarters = [t0 t1 | t2 t3 | t4 t5 | t6 t7]
TWIST_SIGN = [+1, +1, -1, -1, -1, -1, +1, +1]

_CACHE = {}


def _build_graph():
    import concourse.bass as bass
    import concourse.bacc as bacc
    import concourse.tile as tile
    import concourse.mybir as mybir

    f32 = mybir.dt.float32
    bf16 = mybir.dt.bfloat16
    AF = mybir.ActivationFunctionType
    ALU = mybir.AluOpType
    AX = mybir.AxisListType

    nc = bacc.Bacc(
        "TRN2", target_bir_lowering=False, debug=False, num_devices=NCORES
    )

    # ---- DRAM parameters (per-core shard; ALL tensors are host-packed into
    # their exact SBUF tile layouts so every DMA is partition-contiguous:
    # one DMA = one HW channel, and small strided rows run descriptor-bound)
    f8 = mybir.dt.float8e4
    DR = mybir.MatmulPerfMode.DoubleRow
    # x carrier / fp8 copy: [p, chunk, slab, tok]
    x_d = nc.declare_dram_parameter(
        "x", [128, NCHUNK, 8, CHUNK], bf16, isOutput=False)
    x8_d = nc.declare_dram_parameter(
        "x8", [128, NCHUNK, 8, CHUNK], f8, isOutput=False)
    # the partner core's fp8 shard: the seq-mean needs the other half of the
    # batch row; summing the replicated shard locally replaces a ~19us-latency
    # pairwise AllReduce
    x8p_d = nc.declare_dram_parameter(
        "x8p", [128, NCHUNK, 8, CHUNK], f8, isOutput=False)
    # coupling-net weights fp8 (DoubleRow layout): [p, fo, s, j] with
    # w1f[p, fo, s, j] == w1[s*128+p, fo*128+j]
    w1_d = nc.declare_dram_parameter("cn_w1", [128, 8, 8, 128], f8, isOutput=False)
    w2_d = nc.declare_dram_parameter("cn_w2", [128, 4, 8, 128], f8, isOutput=False)
    w3_d = nc.declare_dram_parameter("cn_w3", [128, 2, 4, 128], f8, isOutput=False)
    w4_d = nc.declare_dram_parameter("cn_w4", [128, 2, 1], f8, isOutput=False)
    # all biases + scalars packed into one small tensor (single DMA):
    # cols 0-7 b1, 8-11 b2, 12-13 b3, 14-17 gb1, 18-19 gb2;
    # partition-0 scalars: [0,20]=b4/2 [0,21]=gb3/2 [0,22]=adaptive_range
    cst_d = nc.declare_dram_parameter("consts", [128, 25], f32, isOutput=False)
    # global-net weights bf16: [p, k, m] with gw1[p, k, m] == gc_w1[k*128+p, m]
    gw1_d = nc.declare_dram_parameter("gc_w1", [128, 8, 512], bf16, isOutput=False)
    gw2_d = nc.declare_dram_parameter("gc_w2", [128, 4, 256], bf16, isOutput=False)
    gw3_d = nc.declare_dram_parameter("gc_w3", [128, 2, 1], bf16, isOutput=False)
    out_d = nc.declare_dram_parameter(
        "out", [128, NCHUNK, 8, CHUNK], bf16, isOutput=True)

    with tile.TileContext(nc) as tc:
        with (
            tc.tile_pool(name="const", bufs=1) as const,
            tc.tile_pool(name="work", bufs=2) as work,
            tc.tile_pool(name="xbp", bufs=3) as xbp,
            tc.tile_pool(name="psm", bufs=4, space="PSUM") as psm,
            tc.tile_pool(name="psx", bufs=1, space="PSUM") as psx,
            tc.tile_pool(name="psl4", bufs=2, space="PSUM") as psl4,
            tc.tile_pool(name="pscb", bufs=1, space="PSUM") as pscb,
            tc.tile_pool(name="dram", bufs=1, space="DRAM") as dram,
        ):
            # dma_start ISSUE costs ~0.6us on a sequencer, serially.  Only
            # sync has the fast HWDGE path for bulk; the ACT sequencer is idle
            # for the first ~15us, so it issues the const/weight DMAs, letting
            # sync start on x immediately.
            early = [0]

            def dma_rr(out, in_):
                if early[0] > 0:
                    early[0] -= 1
                    nc.scalar.dma_start(out=out, in_=in_)
                else:
                    nc.sync.dma_start(out=out, in_=in_)

            # bias/constant tile first (a late bias DMA gates every GELU on
            # the in-order ACT queue); one packed DMA.
            early[0] = 8  # cst + w1f quarters + w4 + w2f + w3f
            cst = const.tile([128, 25], f32, tag="cst")
            dma_rr(cst[:], cst_d[:, :])
            b1 = cst[:, 0:8]
            b2 = cst[:, 8:12]
            b3 = cst[:, 12:14]
            gb1 = cst[:, 14:18]
            gb2 = cst[:, 18:20]
            b4h = cst[0:1, 20:21]   # b4 / 2 (host-packed)
            gb3h = cst[0:1, 21:22]  # gb3 / 2 (host-packed)
            ar = cst[0:1, 22:23]
            ar7 = cst[0:1, 23:24]   # 0.7 * adaptive_range (host-packed)
            ar3 = cst[0:1, 24:25]   # 0.3 * adaptive_range (host-packed)

            # prime the ACT table with the gelu set before anything else so
            # the implicit reload (1.28us) doesn't land in front of the first
            # real GELU mid-startup
            prime = const.tile([1, 1], f32, tag="prime")
            nc.vector.memset(prime[:], 0.0)
            prime2 = const.tile([1, 1], f32, tag="prime2")
            nc.scalar.activation(prime2[:], prime[:], AF.Gelu)

            # coupling-net weights: only these 6 issues sit ahead of the
            # gelus in the ACT queue (HWDGE ring-full backpressure stalls
            # the issuing sequencer, and every op behind it)
            w1f = const.tile([128, 8, 8, 128], f8, tag="w1f")  # [p, fo, s, j]
            for h in range(4):
                dma_rr(w1f[:, 2 * h:2 * h + 2, :, :],
                       w1_d[:, 2 * h:2 * h + 2, :, :])
            w4f = const.tile([128, 2, 1], f8, tag="w4f")
            dma_rr(w4f[:], w4_d[:, :, :])
            w2f = const.tile([128, 4, 8, 128], f8, tag="w2f")
            dma_rr(w2f[:], w2_d[:, :, :, :])
            w3f = const.tile([128, 2, 4, 128], f8, tag="w3f")
            dma_rr(w3f[:], w3_d[:, :, :, :])
            gw1 = const.tile([128, 8, 512], bf16, tag="gw1")  # [p, k, m]
            gw2 = const.tile([128, 4, 256], bf16, tag="gw2")
            gw3 = const.tile([128, 2, 1], bf16, tag="gw3")

            def load_gw():
                dma_rr(gw1[:], gw1_d[:, :, :])
                dma_rr(gw2[:], gw2_d[:, :, :])
                dma_rr(gw3[:], gw3_d[:, :, :])

            # x carrier (bf16, updated in place) + cycle-0 fp8 moving operand;
            # two DMAs per chunk each (slab halves -> separate HW channels),
            # interleaved so chunk 0 lands first.
            xc = []   # [128, slab, tok] bf16 — `out` lives here
            x8c = []  # [128, slab, tok] fp8
            # all x8 first: the fp8 copy feeds both the cycle-0 matmuls AND
            # the global-context partial sums, so the collective can launch
            # ~20us in; the bf16 carrier is only read by the twist updates
            # (~55us+) and follows on the same ring.
            # ~36 GB/s per DMA channel, 2-4 concurrent per ring: fine
            # pieces raise ring concurrency and cut chunk latency
            for c in range(NCHUNK):
                x8t = const.tile([128, 8, CHUNK], f8, tag=f"x8_{c}")
                x8c.append(x8t)
                nsplit = 4 if c < 2 else 2
                w = 8 // nsplit
                for h in range(nsplit):
                    sl = slice(w * h, w * h + w)
                    nc.sync.dma_start(out=x8t[:, sl, :], in_=x8_d[:, c, sl, :])
            # the bf16 carrier is only read by the twist updates (~60us+):
            # chunks 0,1 ride the scalar ring behind the weights; chunks 2,3
            # ride the sync ring behind the collective DMAs (issues emitted
            # later so they never block those).  Tiles declared here.
            for c in range(NCHUNK):
                xt = const.tile([128, 8, CHUNK], bf16, tag=f"xc_{c}")
                xc.append(xt)

            def load_xc(c, eng):
                for h in range(2):
                    sl = slice(4 * h, 4 * h + 4)
                    eng.dma_start(out=xc[c][:, sl, :], in_=x_d[:, c, sl, :])

            x8p = []
            for c in range(NCHUNK):
                x8pt = const.tile([128, 8, CHUNK], f8, tag=f"x8p_{c}")
                x8p.append(x8pt)

            def load_x8p(c, eng):
                for h in range(2):
                    sl = slice(4 * h, 4 * h + 4)
                    eng.dma_start(out=x8p[c][:, sl, :], in_=x8p_d[:, c, sl, :])

            def out_bf(t, c):
                return xc[c][:, t, :]

            # per-chunk tanh tiles + coupling-broadcast operands.  The
            # coupling affine c0' + cmul'*th rides the PE broadcast as TWO
            # accumulating K=1 matmuls (engines cannot write partition 1, so
            # a single K=2 stationary cannot be built from runtime scalars):
            #   cb = (cmul'*ones) (x) th  +  (c0'*ones) (x) ones_row
            th_t = []
            for c in range(NCHUNK):
                th = const.tile([2, CHUNK], bf16, tag=f"th_{c}")
                # row 0 is overwritten with tanh by ACT each cycle; row 1
                # stays 1.0 (memset must start at partition 0, so set both)
                nc.vector.memset(th[0:2, :], 1.0)
                th_t.append(th)
            cm2 = const.tile([2, 128], bf16, tag="cm2")
            c0_row = const.tile([1, 128], bf16, tag="c0_row")
            cm2row0 = [False]
            ones = const.tile([1, 128], bf16, tag="ones")
            nc.vector.memset(ones[:], 1.0)
            ones_row = const.tile([1, CHUNK], bf16, tag="ones_row")
            nc.vector.memset(ones_row[:], 1.0)
            # cm2 row 0 (cmul' = 0.3*ar) is input-only: built at startup, off
            # the gc critical chain; row 1 (runtime c0') lands via DMA later
            nc.vector.tensor_scalar(
                cm2[0:1, :], ones[:], ar3, None, ALU.mult)
            bcoup = const.tile([1, 1], f32, tag="bcoup")
            nc.vector.memset(bcoup[:], BASE_COUPLING)

            # ---------------- global-context partial sums + AllReduce -------
            # per-(tile, chunk) partial sums on DVE (idle during cycle 0);
            # emitted per-chunk so the in-order queue never blocks on later x.
            red = const.tile([128, 8, NCHUNK], f32, tag="gred")

            def reduce_chunk(c):
                # partial sums from the fp8 copy: ~4% error on the seq-mean,
                # heavily damped downstream (gf enters the coupling at 0.14x
                # through a saturating tanh); buys ~25us of collective lead
                for t in range(8):
                    nc.vector.tensor_reduce(
                        red[:, t, c:c + 1], x8c[c][:, t, :], axis=AX.X,
                        op=ALU.add
                    )

            redp = const.tile([128, 8, NCHUNK], f32, tag="gredp")

            def reduce_partner(c):
                for t in range(8):
                    nc.vector.tensor_reduce(
                        redp[:, t, c:c + 1], x8p[c][:, t, :], axis=AX.X,
                        op=ALU.add
                    )

            gs = const.tile([128, 8], f32, tag="gs")
            gsp = const.tile([128, 8], f32, tag="gsp")
            gmean = const.tile([128, 8], bf16, tag="gmean")

            def finish_gsum():
                for t in range(8):
                    nc.vector.tensor_reduce(
                        gs[:, t:t + 1], red[:, t, :], axis=AX.X, op=ALU.add
                    )
                    nc.vector.tensor_reduce(
                        gsp[:, t:t + 1], redp[:, t, :], axis=AX.X, op=ALU.add
                    )
                # full-seq sum = own half + replicated partner half
                nc.vector.tensor_add(gs[:], gs[:], gsp[:])
                nc.vector.tensor_copy(gmean[:], gs[:])

            # ---------------- global net (emitted via hooks; see cycle 0) ---
            gc_tiles = {}

            def gc_stage1():
                # all 4 output-tile groups accumulate into one PSUM bank
                # (disjoint columns) -> a single GELU epilogue
                ps = psx.tile([128, 4], f32, tag="aux")
                for fo in range(4):
                    for k in range(8):
                        nc.tensor.matmul(
                            ps[:, fo:fo + 1],
                            gw1[:, k, fo * 128:(fo + 1) * 128],
                            gmean[:, k:k + 1], start=(k == 0), stop=(k == 7),
                        )
                # psum holds gc_w1.T @ sum(x); fold the 1/S mean + bias on DVE
                # (activation bias APs must be [P,1]; gb1 varies per column)
                z1 = work.tile([128, 4], f32, tag="z1")
                nc.vector.scalar_tensor_tensor(
                    z1[:], ps[:], 1.0 / S, gb1, ALU.mult, ALU.add
                )
                g1 = work.tile([128, 4], bf16, tag="g1")
                nc.scalar.activation(g1[:], z1[:], AF.Gelu)
                gc_tiles["g1"] = g1

            def gc_stage2():
                g1 = gc_tiles["g1"]
                ps = psx.tile([128, 2], f32, tag="aux")
                for fo in range(2):
                    for k in range(4):
                        nc.tensor.matmul(
                            ps[:, fo:fo + 1],
                            gw2[:, k, fo * 128:(fo + 1) * 128],
                            g1[:, k:k + 1], start=(k == 0), stop=(k == 3),
                        )
                z2 = work.tile([128, 2], f32, tag="z2")
                nc.vector.tensor_add(z2[:], ps[:], gb2)
                g2 = work.tile([128, 2], bf16, tag="g2")
                nc.scalar.activation(g2[:], z2[:], AF.Gelu)
                gc_tiles["g2"] = g2

            def gc_stage3():
                g2 = gc_tiles["g2"]
                ps = psx.tile([1, 1], f32, tag="aux")
                for k in range(2):
                    nc.tensor.matmul(
                        ps[:], gw3[:, k, :], g2[:, k:k + 1],
                        start=(k == 0), stop=(k == 1)
                    )
                # gf_t = tanh((gz + gb3)/2); sigmoid folded into the affine
                gft = const.tile([1, 1], f32, tag="gft")
                nc.scalar.activation(gft[:], ps[:], AF.Tanh, bias=gb3h, scale=0.5)

                # coupling = c0' + cmul' * tanh((z4+b4)/2)
                #   cmul' = 0.3*ar (host-packed, startup) ;
                #   c0'   = 0.1 + (0.7*ar)*gf_t  — one ACT op (Identity is in
                #   the gelu table), then one DVE broadcast + the row-1 DMA
                c0 = const.tile([1, 1], f32, tag="c0")
                nc.scalar.activation(
                    c0[:], gft[:], AF.Identity, bias=bcoup[:], scale=ar7)
                nc.scalar.activation(
                    c0_row[:], ones[:], AF.Identity, bias=0.0, scale=c0[:])
                nc.sync.dma_start(out=cm2[1:2, :], in_=c0_row[:])

            # ---------------- per-chunk building blocks ----------------
            pending_xb = [x8c[c] for c in range(NCHUNK)]
            hstate = {}

            def emit_L1(c, s_order=(0, 1, 2, 3)):
                # s_order (0,2,1,3) for post-update cycles: the twist pairs
                # produce xb slabs {p, p+4}, so K-groups are consumed in the
                # order the casts retire; cycle 0's x8 DMAs arrive in slab
                # order, so the natural order is kept there.
                xb = pending_xb[c]
                pending_xb[c] = None
                h1 = work.tile([128, 8, CHUNK], f8, tag="h1")
                for fo in range(8):
                    ps1 = psm.tile([128, CHUNK], f32, tag="mm")
                    for si, s in enumerate(s_order):
                        nc.tensor.matmul(
                            ps1[:], w1f[:, fo, 2 * s:2 * s + 2, :],
                            xb[:, 2 * s:2 * s + 2, :],
                            start=(si == 0), stop=(si == 3), perf_mode=DR,
                        )
                    nc.scalar.activation(
                        h1[:, fo, :], ps1[:], AF.Gelu, bias=b1[:, fo:fo + 1])
                hstate[c] = h1

            def emit_L2(c):
                h1 = hstate[c]
                h2 = work.tile([128, 4, CHUNK], f8, tag="h2")
                for fo in range(4):
                    ps2 = psm.tile([128, CHUNK], f32, tag="mm")
                    for s in range(4):
                        nc.tensor.matmul(
                            ps2[:], w2f[:, fo, 2 * s:2 * s + 2, :],
                            h1[:, 2 * s:2 * s + 2, :],
                            start=(s == 0), stop=(s == 3), perf_mode=DR,
                        )
                    nc.scalar.activation(
                        h2[:, fo, :], ps2[:], AF.Gelu, bias=b2[:, fo:fo + 1])
                hstate[c] = h2

            def emit_L34(c):
                h2 = hstate.pop(c)
                h3 = work.tile([128, 2, CHUNK], f8, tag="h3")
                for fo in range(2):
                    ps3 = psm.tile([128, CHUNK], f32, tag="mm")
                    for s in range(2):
                        nc.tensor.matmul(
                            ps3[:], w3f[:, fo, 2 * s:2 * s + 2, :],
                            h2[:, 2 * s:2 * s + 2, :],
                            start=(s == 0), stop=(s == 1), perf_mode=DR,
                        )
                    nc.scalar.activation(
                        h3[:, fo, :], ps3[:], AF.Gelu, bias=b3[:, fo:fo + 1])
                # L4: M=1 forbids the DoubleRow ldweights layout -> 2 plain
                # fp8 matmuls (ISA check s3_lw_dual_fp8_restrictions)
                ps4 = psl4.tile([1, CHUNK], f32, tag="l4")
                for s in range(2):
                    nc.tensor.matmul(
                        ps4[:], w4f[:, s, :], h3[:, s, :],
                        start=(s == 0), stop=(s == 1),
                    )
                # th = tanh((z4 + b4)/2); sigmoid folded into the coupling
                nc.scalar.activation(
                    th_t[c][0:1, :], ps4[:], AF.Tanh, bias=b4h, scale=0.5)

            def mlp_chunk(c, hooks=()):
                """coupling-net MLP on chunk c of `out`.

                hooks: up to 3 closures emitted after L1/L2/L3+L4 — used to
                slot the previous chunks' coupling-broadcast + twist updates
                (and the tiny serial gc-net chain at cycle 0) into the queues
                at points where their ACT/DVE dependencies have had time to
                finish.
                """
                hooks = list(hooks) + [None] * 3
                emit_L1(c, s_order=(0, 2, 1, 3))
                if hooks[0]:
                    hooks[0]()
                emit_L2(c)
                if hooks[1]:
                    hooks[1]()
                emit_L34(c)
                if hooks[2]:
                    hooks[2]()

            def update_chunk(c, last, next_conv=False, k1=False):
                """coupling broadcast + twist update (in place) on chunk c;
                one consolidated DMA out if last.

                k1: two accumulating K=1 matmuls instead of the K=2 form —
                used for the first cycle-0 updates, which would otherwise
                wait on the cm2 row-1 SBUF->SBUF DMA (~2us latency)."""
                cb = pscb.tile([128, CHUNK], f32, tag="cb")
                if k1:
                    nc.tensor.matmul(
                        cb[:], cm2[0:1, :], th_t[c][0:1, :],
                        start=True, stop=False)
                    nc.tensor.matmul(
                        cb[:], c0_row[:], ones_row[:], start=False, stop=True)
                else:
                    # cb[p, j] = cmul'*th[j] + c0'*1  via one K=2 matmul
                    nc.tensor.matmul(
                        cb[:], cm2[:, :], th_t[c][:, :], start=True, stop=True)
                # one bf16 SBUF copy so the twist tensor_tensor ops all run
                # in the DVE 2x_1p fast mode (PSUM/f32 operands disable it)
                cbb = work.tile([128, CHUNK], bf16, tag="cbb")
                nc.vector.tensor_copy(cbb[:], cb[:])
                xb_next = None
                for p in range(4):
                    t, u = p, p + 4
                    tmpa = work.tile([128, CHUNK], bf16, tag="twa")
                    tmpb = work.tile([128, CHUNK], bf16, tag="twb")
                    if next_conv and xb_next is None:
                        xb_next = xbp.tile([128, 8, CHUNK], f8, tag="xb")
                    nc.vector.tensor_mul(tmpa[:], out_bf(u, c), cbb[:])
                    nc.vector.tensor_mul(tmpb[:], out_bf(t, c), cbb[:])
                    if TWIST_SIGN[t] > 0:
                        nc.vector.tensor_add(out_bf(t, c), out_bf(t, c), tmpa[:])
                    else:
                        nc.vector.tensor_sub(out_bf(t, c), out_bf(t, c), tmpa[:])
                    if next_conv:
                        # fp8 conversion per slab, immediately after its add:
                        # the next chunk's L1 K-groups unblock ~1us earlier
                        nc.vector.tensor_copy(xb_next[:, t, :], out_bf(t, c))
                    if TWIST_SIGN[u] > 0:
                        nc.vector.tensor_add(out_bf(u, c), out_bf(u, c), tmpb[:])
                    else:
                        nc.vector.tensor_sub(out_bf(u, c), out_bf(u, c), tmpb[:])
                    if next_conv:
                        nc.vector.tensor_copy(xb_next[:, u, :], out_bf(u, c))
                    if last and p % 2 == 1:
                        # pairs 0,1 finalize slabs {0,1,4,5}; pairs 2,3 the
                        # rest: ship each contiguous quarter as soon as its
                        # twist adds retire, one per HWDGE ring, so the
                        # transfers overlap the remaining update ops
                        lo = p - 1
                        nc.scalar.dma_start(
                            out=out_d[:, c, lo:lo + 2, :],
                            in_=xc[c][:, lo:lo + 2, :])
                        nc.sync.dma_start(
                            out=out_d[:, c, lo + 4:lo + 6, :],
                            in_=xc[c][:, lo + 4:lo + 6, :])
                if next_conv and not last:
                    pending_xb[c] = xb_next

            # ---------------- main cycles ----------------
            # Cycle 0, chunks 0+1 layer-paired: the in-order PE queue would
            # otherwise stall L1(1) behind L2(0)'s wait for chunk-0's trailing
            # GELU at the very start (ACT lags the first chunk's matmuls).
            # Reductions are emitted eagerly; the collective is issued as soon
            # as chunk 3's x lands; the serial gc-net stages ride mlp(2)'s
            # hooks (gmean arrives ~30us, well before); cycle-0 updates then
            # ride mlp(3)'s hooks so cycle-1 L1s start with zero boundary
            # stall.  From there every mlp hook emits the oldest pending
            # update (lag-2 at the boundary, catching back to lag-1 in cycle
            # 1 so the tail stays one update deep).
            load_x8p(0, nc.sync)
            load_x8p(1, nc.sync)
            load_x8p(2, nc.sync)
            load_x8p(3, nc.sync)
            emit_L1(0)
            reduce_chunk(0)
            emit_L2(0)
            emit_L34(0)
            emit_L1(1)
            reduce_chunk(1)
            reduce_chunk(2)
            reduce_chunk(3)
            reduce_partner(0)
            reduce_partner(1)
            reduce_partner(2)
            reduce_partner(3)
            finish_gsum()
            early[0] = 3
            load_gw()
            emit_L2(1)
            emit_L1(2)
            load_xc(0, nc.scalar)
            load_xc(1, nc.scalar)
            emit_L34(1)
            emit_L2(2)
            gc_stage1()
            emit_L1(3)
            gc_stage2()
            emit_L34(2)
            gc_stage3()

            def upd(c, last=False, k1=False):
                return lambda: update_chunk(
                    c, last, next_conv=not last, k1=k1)

            load_xc(2, nc.scalar)
            load_xc(3, nc.scalar)
            upd(0, k1=True)()
            emit_L2(3)
            upd(1, k1=True)()
            emit_L34(3)
            # cycle 1: catch from lag-2 back to lag-1
            mlp_chunk(0, hooks=(upd(2),))
            mlp_chunk(1, hooks=(upd(3), None, upd(0)))
            mlp_chunk(2, hooks=(upd(1),))
            mlp_chunk(3, hooks=(upd(2),))
            # cycle 2 (last): updates write the final output + DMA out.
            # The last two chunks are layer-paired so the end-of-stream
            # GELU-latency waits are filled with the other chunk's matmuls.
            mlp_chunk(0, hooks=(upd(3),))
            mlp_chunk(1, hooks=(upd(0, last=True),))
            emit_L1(2, s_order=(0, 2, 1, 3))
            upd(1, last=True)()
            emit_L2(2)
            emit_L1(3, s_order=(0, 2, 1, 3))
            emit_L34(2)
            upd(2, last=True)()
            emit_L2(3)
            emit_L34(3)
            update_chunk(3, last=True, next_conv=False)

    nc.compile()
    return nc


def _get_graph():
    if "nc" not in _CACHE:
        _CACHE["nc"] = _build_graph()
    return _CACHE["nc"]


def _pack_consts(inputs):
    cst = np.zeros((128, 25), np.float32)
    cst[:, 0:8] = np.asarray(inputs["cn_b1"], np.float32).reshape(8, 128).T
    cst[:, 8:12] = np.asarray(inputs["cn_b2"], np.float32).reshape(4, 128).T
    cst[:, 12:14] = np.asarray(inputs["cn_b3"], np.float32).reshape(2, 128).T
    cst[:, 14:18] = np.asarray(inputs["gc_b1"], np.float32).reshape(4, 128).T
    cst[:, 18:20] = np.asarray(inputs["gc_b2"], np.float32).reshape(2, 128).T
    cst[0, 20] = 0.5 * np.asarray(inputs["cn_b4"], np.float32).reshape(())
    cst[0, 21] = 0.5 * np.asarray(inputs["gc_b3"], np.float32).reshape(())
    arv = np.asarray(inputs["adaptive_range"], np.float32).reshape(())
    cst[0, 22] = arv
    cst[0, 23] = 0.7 * arv
    cst[0, 24] = 0.3 * arv
    return cst


def _make_in_maps(inputs):
    import ml_dtypes

    bf = ml_dtypes.bfloat16
    f8 = ml_dtypes.float8_e4m3
    x = np.ascontiguousarray(inputs["x"], dtype=np.float32)
    # [core, p, chunk, slab, tok] — exact SBUF tile layout, so every DMA is
    # partition-contiguous: x[b, s, d] with token j = chunk*512+tok,
    # feature f = slab*128+p
    xs = (x.reshape(NCORES, NCHUNK, CHUNK, 8, 128)
          .transpose(0, 4, 1, 3, 2))  # [8, 128, 4, 8, 512]
    xs = np.ascontiguousarray(xs)

    def pack_w(w, nfo):
        # [s*128+p, fo*128+j] -> [p, fo, s, j]
        kin = w.shape[0]
        return np.ascontiguousarray(
            np.asarray(w).reshape(kin // 128, 128, nfo, 128)
            .transpose(1, 2, 0, 3), dtype=f8)

    def pack_gw(w):
        # [k*128+p, m] -> [p, k, m]
        kin, m = w.shape
        return np.ascontiguousarray(
            np.asarray(w).reshape(kin // 128, 128, m).transpose(1, 0, 2),
            dtype=bf)

    shared = {
        "cn_w1": pack_w(inputs["cn_w1"], 8),
        "cn_w2": pack_w(inputs["cn_w2"], 4),
        "cn_w3": pack_w(inputs["cn_w3"], 2),
        "cn_w4": np.ascontiguousarray(
            np.asarray(inputs["cn_w4"]).reshape(2, 128).T.reshape(128, 2, 1),
            dtype=f8),
        "gc_w1": pack_gw(np.asarray(inputs["gc_w1"])),
        "gc_w2": pack_gw(np.asarray(inputs["gc_w2"])),
        "gc_w3": pack_gw(np.asarray(inputs["gc_w3"]).reshape(256, 1)),
        "consts": _pack_consts(inputs),
    }
    xs8 = xs.astype(f8)
    in_maps = []
    for c in range(NCORES):
        m = dict(shared)
        m["x"] = xs[c].astype(bf)
        m["x8"] = xs8[c]
        m["x8p"] = xs8[c ^ 1]  # partner = other half of the same batch row
        in_maps.append(m)
    return in_maps


def _run(inputs, trace=False):
    from concourse.bass_utils import run_bass_kernel_spmd

    nc = _get_graph()
    in_maps = _make_in_maps(inputs)
    res = run_bass_kernel_spmd(
        nc, in_maps, core_ids=list(range(NCORES)), trace=trace
    )
    # out[p, chunk, slab, tok] -> [token = chunk*512+tok, feat = slab*128+p]
    outs = np.stack(
        [np.asarray(res.results[c]["out"]).astype(np.float32)
         .transpose(1, 3, 2, 0).reshape(TOK, DIM)
         for c in range(NCORES)], axis=0
    )  # [8, 2048, 1024]
    full = outs.reshape(B, S, DIM).astype(np.float32)
    return full, res


def kernel(**inputs) -> np.ndarray:
    out, _ = _run(inputs, trace=False)
    return out


# revision 37
# speedup vs baseline: 1.0077x; 1.0077x over previous
"""Trainium2 Bass kernel for nn_AdaptiveMobiusLayer.

Strategy (pure data parallel over tokens, 8 NeuronCores):
  - Flatten x [4, 4096, 1024] -> [16384, 1024] tokens; core c takes 2048
    consecutive tokens (= batch b = c//2, seq half c%2).
  - Host transposes each shard to [1024 feats, 2048 tokens] and ships it
    twice: bf16 (the in-SBUF `out` carrier) and fp8 (cycle-0 matmul moving
    operand), so the device does no cycle-0 conversions.
  - Feature-major tiles: every matmul keeps features on partitions (weights
    are natural [K, M] lhsT stationary operands, activations moving).
  - The seq-mean for the global context needs the partner core's half of the
    batch row: the partner's fp8 shard is replicated to each core and summed
    locally during cycle 0 — replacing a pairwise AllReduce whose end-to-end
    latency (~19us) dominated the cycle-0 critical path.
  - MLP matmuls run in fp8 DoubleRow (fp32 accumulation in PSUM); `out` is
    carried in bf16 so the twist-update tensor_tensor ops hit the DVE 2x_1p
    fast path.
  - All sigmoids are computed as 0.5 + 0.5*tanh(z/2); the coupling affine
    c0' + cmul'*tanh rides the partition-broadcast as two accumulating K=1
    matmuls, so ACT only ever uses the gelu table (gelu/tanh share it -> no
    ACT_TABLE_LOAD thrash):
      coupling = (0.1 + 0.7*ar*gf_t) + (0.3*ar)*tanh((z4+b4)/2),
      gf_t = tanh((gz+gb3)/2).
  - Emission order software-pipelines the in-order engine queues: cycle-0
    layers staggered across chunks (DMA delivery pacing), gc-net stages and
    twist updates hooked between MLP layers, updates lag-2 across the
    cycle-0/1 boundary catching back to lag-1 so the tail stays one update
    deep.  All DRAM tensors are host-packed into exact SBUF tile layouts so
    every DMA row is partition-contiguous (dma_start issue ~0.6us sequencer
    time each; ~36-70 GB/s per channel, ~2 concurrent per HWDGE ring).
"""

import sys

sys.path.insert(0, "/opt/trn_rl_repo")

import numpy as np

B, S, DIM = 4, 4096, 1024
NCORES = 8
TOK = B * S // NCORES  # 2048 tokens per core
CHUNK = 512
NCHUNK = TOK // CHUNK  # 4
NUM_CYCLES = 3
BASE_COUPLING = 0.1

# feature-quarter twist:  out_new[t] = out[t] + sign[t] * c * out[(t+4) % 8]
# tiles 0..7 are 128-feature slabs; quarters = [t0 t1 | t2 t3 | t4 t5 | t6 t7]
TWIST_SIGN = [+1, +1, -1, -1, -1, -1, +1, +1]

_CACHE = {}


def _build_graph():
    import concourse.bass as bass
    import concourse.bacc as bacc
    import concourse.tile as tile
    import concourse.mybir as mybir

    f32 = mybir.dt.float32
    bf16 = mybir.dt.bfloat16
    AF = mybir.ActivationFunctionType
    ALU = mybir.AluOpType
    AX = mybir.AxisListType

    nc = bacc.Bacc(
        "TRN2", target_bir_lowering=False, debug=False, num_devices=NCORES
    )

    # ---- DRAM parameters (per-core shard; ALL tensors are host-packed into
    # their exact SBUF tile layouts so every DMA is partition-contiguous:
    # one DMA = one HW channel, and small strided rows run descriptor-bound)
    f8 = mybir.dt.float8e4
    DR = mybir.MatmulPerfMode.DoubleRow
    # x carrier / fp8 copy: [p, chunk, slab, tok]
    x_d = nc.declare_dram_parameter(
        "x", [128, NCHUNK, 8, CHUNK], bf16, isOutput=False)
    x8_d = nc.declare_dram_parameter(
        "x8", [128, NCHUNK, 8, CHUNK], f8, isOutput=False)
    # the partner core's fp8 shard: the seq-mean needs the other half of the
    # batch row; summing the replicated shard locally replaces a ~19us-latency
    # pairwise AllReduce
    x8p_d = nc.declare_dram_parameter(
        "x8p", [128, NCHUNK, 8, CHUNK], f8, isOutput=False)
    # coupling-net weights fp8 (DoubleRow layout): [p, fo, s, j] with
    # w1f[p, fo, s, j] == w1[s*128+p, fo*128+j]
    w1_d = nc.declare_dram_parameter("cn_w1", [128, 8, 8, 128], f8, isOutput=False)
    w2_d = nc.declare_dram_parameter("cn_w2", [128, 4, 8, 128], f8, isOutput=False)
    w3_d = nc.declare_dram_parameter("cn_w3", [128, 2, 4, 128], f8, isOutput=False)
    w4_d = nc.declare_dram_parameter("cn_w4", [128, 2, 1], f8, isOutput=False)
    # all biases + scalars packed into one small tensor (single DMA):
    # cols 0-7 b1, 8-11 b2, 12-13 b3, 14-17 gb1, 18-19 gb2;
    # partition-0 scalars: [0,20]=b4/2 [0,21]=gb3/2 [0,22]=adaptive_range
    cst_d = nc.declare_dram_parameter("consts", [128, 25], f32, isOutput=False)
    # global-net weights bf16: [p, k, m] with gw1[p, k, m] == gc_w1[k*128+p, m]
    gw1_d = nc.declare_dram_parameter("gc_w1", [128, 8, 512], bf16, isOutput=False)
    gw2_d = nc.declare_dram_parameter("gc_w2", [128, 4, 256], bf16, isOutput=False)
    gw3_d = nc.declare_dram_parameter("gc_w3", [128, 2, 1], bf16, isOutput=False)
    out_d = nc.declare_dram_parameter(
        "out", [128, NCHUNK, 8, CHUNK], bf16, isOutput=True)

    with tile.TileContext(nc) as tc:
        with (
            tc.tile_pool(name="const", bufs=1) as const,
            tc.tile_pool(name="work", bufs=2) as work,
            tc.tile_pool(name="xbp", bufs=3) as xbp,
            tc.tile_pool(name="psm", bufs=4, space="PSUM") as psm,
            tc.tile_pool(name="psx", bufs=1, space="PSUM") as psx,
            tc.tile_pool(name="psl4", bufs=2, space="PSUM") as psl4,
            tc.tile_pool(name="pscb", bufs=1, space="PSUM") as pscb,
            tc.tile_pool(name="dram", bufs=1, space="DRAM") as dram,
        ):
            # dma_start ISSUE costs ~0.6us on a sequencer, serially.  Only
            # sync has the fast HWDGE path for bulk; the ACT sequencer is idle
            # for the first ~15us, so it issues the const/weight DMAs, letting
            # sync start on x immediately.
            early = [0]

            def dma_rr(out, in_):
                if early[0] > 0:
                    early[0] -= 1
                    nc.scalar.dma_start(out=out, in_=in_)
                else:
                    nc.sync.dma_start(out=out, in_=in_)

            # bias/constant tile first (a late bias DMA gates every GELU on
            # the in-order ACT queue); one packed DMA.
            early[0] = 6  # cst + w1f halves + w4 + w2f + w3f
            cst = const.tile([128, 25], f32, tag="cst")
            dma_rr(cst[:], cst_d[:, :])
            b1 = cst[:, 0:8]
            b2 = cst[:, 8:12]
            b3 = cst[:, 12:14]
            gb1 = cst[:, 14:18]
            gb2 = cst[:, 18:20]
            b4h = cst[0:1, 20:21]   # b4 / 2 (host-packed)
            gb3h = cst[0:1, 21:22]  # gb3 / 2 (host-packed)
            ar = cst[0:1, 22:23]
            ar7 = cst[0:1, 23:24]   # 0.7 * adaptive_range (host-packed)
            ar3 = cst[0:1, 24:25]   # 0.3 * adaptive_range (host-packed)

            # prime the ACT table with the gelu set before anything else so
            # the implicit reload (1.28us) doesn't land in front of the first
            # real GELU mid-startup
            prime = const.tile([1, 1], f32, tag="prime")
            nc.vector.memset(prime[:], 0.0)
            prime2 = const.tile([1, 1], f32, tag="prime2")
            nc.scalar.activation(prime2[:], prime[:], AF.Gelu)

            # coupling-net weights: only these 6 issues sit ahead of the
            # gelus in the ACT queue (HWDGE ring-full backpressure stalls
            # the issuing sequencer, and every op behind it)
            w1f = const.tile([128, 8, 8, 128], f8, tag="w1f")  # [p, fo, s, j]
            for h in range(2):
                dma_rr(w1f[:, 4 * h:4 * h + 4, :, :],
                       w1_d[:, 4 * h:4 * h + 4, :, :])
            w4f = const.tile([128, 2, 1], f8, tag="w4f")
            dma_rr(w4f[:], w4_d[:, :, :])
            w2f = const.tile([128, 4, 8, 128], f8, tag="w2f")
            dma_rr(w2f[:], w2_d[:, :, :, :])
            w3f = const.tile([128, 2, 4, 128], f8, tag="w3f")
            dma_rr(w3f[:], w3_d[:, :, :, :])
            gw1 = const.tile([128, 8, 512], bf16, tag="gw1")  # [p, k, m]
            gw2 = const.tile([128, 4, 256], bf16, tag="gw2")
            gw3 = const.tile([128, 2, 1], bf16, tag="gw3")

            def load_gw():
                dma_rr(gw1[:], gw1_d[:, :, :])
                dma_rr(gw2[:], gw2_d[:, :, :])
                dma_rr(gw3[:], gw3_d[:, :, :])

            # x carrier (bf16, updated in place) + cycle-0 fp8 moving operand;
            # two DMAs per chunk each (slab halves -> separate HW channels),
            # interleaved so chunk 0 lands first.
            xc = []   # [128, slab, tok] bf16 — `out` lives here
            x8c = []  # [128, slab, tok] fp8
            # all x8 first: the fp8 copy feeds both the cycle-0 matmuls AND
            # the global-context partial sums, so the collective can launch
            # ~20us in; the bf16 carrier is only read by the twist updates
            # (~55us+) and follows on the same ring.
            # ~36 GB/s per DMA channel, 2-4 concurrent per ring: fine
            # pieces raise ring concurrency and cut chunk latency
            for c in range(NCHUNK):
                x8t = const.tile([128, 8, CHUNK], f8, tag=f"x8_{c}")
                x8c.append(x8t)
                nsplit = 4 if c < 2 else 2
                w = 8 // nsplit
                for h in range(nsplit):
                    sl = slice(w * h, w * h + w)
                    nc.sync.dma_start(out=x8t[:, sl, :], in_=x8_d[:, c, sl, :])
            # the bf16 carrier is only read by the twist updates (~60us+):
            # chunks 0,1 ride the scalar ring behind the weights; chunks 2,3
            # ride the sync ring behind the collective DMAs (issues emitted
            # later so they never block those).  Tiles declared here.
            for c in range(NCHUNK):
                xt = const.tile([128, 8, CHUNK], bf16, tag=f"xc_{c}")
                xc.append(xt)

            def load_xc(c, eng):
                for h in range(2):
                    sl = slice(4 * h, 4 * h + 4)
                    eng.dma_start(out=xc[c][:, sl, :], in_=x_d[:, c, sl, :])

            x8p = []
            for c in range(NCHUNK):
                x8pt = const.tile([128, 8, CHUNK], f8, tag=f"x8p_{c}")
                x8p.append(x8pt)

            def load_x8p(c, eng):
                for h in range(2):
                    sl = slice(4 * h, 4 * h + 4)
                    eng.dma_start(out=x8p[c][:, sl, :], in_=x8p_d[:, c, sl, :])

            def out_bf(t, c):
                return xc[c][:, t, :]

            # per-chunk tanh tiles + coupling-broadcast operands.  The
            # coupling affine c0' + cmul'*th rides the PE broadcast as TWO
            # accumulating K=1 matmuls (engines cannot write partition 1, so
            # a single K=2 stationary cannot be built from runtime scalars):
            #   cb = (cmul'*ones) (x) th  +  (c0'*ones) (x) ones_row
            th_t = []
            for c in range(NCHUNK):
                th = const.tile([2, CHUNK], bf16, tag=f"th_{c}")
                # row 0 is overwritten with tanh by ACT each cycle; row 1
                # stays 1.0 (memset must start at partition 0, so set both)
                nc.vector.memset(th[0:2, :], 1.0)
                th_t.append(th)
            cm2 = const.tile([2, 128], bf16, tag="cm2")
            c0_row = const.tile([1, 128], bf16, tag="c0_row")
            cm2row0 = [False]
            ones = const.tile([1, 128], bf16, tag="ones")
            nc.vector.memset(ones[:], 1.0)
            ones_row = const.tile([1, CHUNK], bf16, tag="ones_row")
            nc.vector.memset(ones_row[:], 1.0)
            # cm2 row 0 (cmul' = 0.3*ar) is input-only: built at startup, off
            # the gc critical chain; row 1 (runtime c0') lands via DMA later
            nc.vector.tensor_scalar(
                cm2[0:1, :], ones[:], ar3, None, ALU.mult)
            bcoup = const.tile([1, 1], f32, tag="bcoup")
            nc.vector.memset(bcoup[:], BASE_COUPLING)

            # ---------------- global-context partial sums + AllReduce -------
            # per-(tile, chunk) partial sums on DVE (idle during cycle 0);
            # emitted per-chunk so the in-order queue never blocks on later x.
            red = const.tile([128, 8, NCHUNK], f32, tag="gred")

            def reduce_chunk(c):
                # partial sums from the fp8 copy: ~4% error on the seq-mean,
                # heavily damped downstream (gf enters the coupling at 0.14x
                # through a saturating tanh); buys ~25us of collective lead
                for t in range(8):
                    nc.vector.tensor_reduce(
                        red[:, t, c:c + 1], x8c[c][:, t, :], axis=AX.X,
                        op=ALU.add
                    )

            redp = const.tile([128, 8, NCHUNK], f32, tag="gredp")

            def reduce_partner(c):
                for t in range(8):
                    nc.vector.tensor_reduce(
                        redp[:, t, c:c + 1], x8p[c][:, t, :], axis=AX.X,
                        op=ALU.add
                    )

            gs = const.tile([128, 8], f32, tag="gs")
            gsp = const.tile([128, 8], f32, tag="gsp")
            gmean = const.tile([128, 8], bf16, tag="gmean")

            def finish_gsum():
                for t in range(8):
                    nc.vector.tensor_reduce(
                        gs[:, t:t + 1], red[:, t, :], axis=AX.X, op=ALU.add
                    )
                    nc.vector.tensor_reduce(
                        gsp[:, t:t + 1], redp[:, t, :], axis=AX.X, op=ALU.add
                    )
                # full-seq sum = own half + replicated partner half
                nc.vector.tensor_add(gs[:], gs[:], gsp[:])
                nc.vector.tensor_copy(gmean[:], gs[:])

            # ---------------- global net (emitted via hooks; see cycle 0) ---
            gc_tiles = {}

            def gc_stage1():
                # all 4 output-tile groups accumulate into one PSUM bank
                # (disjoint columns) -> a single GELU epilogue
                ps = psx.tile([128, 4], f32, tag="aux")
                for fo in range(4):
                    for k in range(8):
                        nc.tensor.matmul(
                            ps[:, fo:fo + 1],
                            gw1[:, k, fo * 128:(fo + 1) * 128],
                            gmean[:, k:k + 1], start=(k == 0), stop=(k == 7),
                        )
                # psum holds gc_w1.T @ sum(x); fold the 1/S mean + bias on DVE
                # (activation bias APs must be [P,1]; gb1 varies per column)
                z1 = work.tile([128, 4], f32, tag="z1")
                nc.vector.scalar_tensor_tensor(
                    z1[:], ps[:], 1.0 / S, gb1, ALU.mult, ALU.add
                )
                g1 = work.tile([128, 4], bf16, tag="g1")
                nc.scalar.activation(g1[:], z1[:], AF.Gelu)
                gc_tiles["g1"] = g1

            def gc_stage2():
                g1 = gc_tiles["g1"]
                ps = psx.tile([128, 2], f32, tag="aux")
                for fo in range(2):
                    for k in range(4):
                        nc.tensor.matmul(
                            ps[:, fo:fo + 1],
                            gw2[:, k, fo * 128:(fo + 1) * 128],
                            g1[:, k:k + 1], start=(k == 0), stop=(k == 3),
                        )
                z2 = work.tile([128, 2], f32, tag="z2")
                nc.vector.tensor_add(z2[:], ps[:], gb2)
                g2 = work.tile([128, 2], bf16, tag="g2")
                nc.scalar.activation(g2[:], z2[:], AF.Gelu)
                gc_tiles["g2"] = g2

            def gc_stage3():
                g2 = gc_tiles["g2"]
                ps = psx.tile([1, 1], f32, tag="aux")
                for k in range(2):
                    nc.tensor.matmul(
                        ps[:], gw3[:, k, :], g2[:, k:k + 1],
                        start=(k == 0), stop=(k == 1)
                    )
                # gf_t = tanh((gz + gb3)/2); sigmoid folded into the affine
                gft = const.tile([1, 1], f32, tag="gft")
                nc.scalar.activation(gft[:], ps[:], AF.Tanh, bias=gb3h, scale=0.5)

                # coupling = c0' + cmul' * tanh((z4+b4)/2)
                #   cmul' = 0.3*ar (host-packed, startup) ;
                #   c0'   = 0.1 + (0.7*ar)*gf_t  — one ACT op (Identity is in
                #   the gelu table), then one DVE broadcast + the row-1 DMA
                c0 = const.tile([1, 1], f32, tag="c0")
                nc.scalar.activation(
                    c0[:], gft[:], AF.Identity, bias=bcoup[:], scale=ar7)
                nc.vector.tensor_scalar(
                    c0_row[:], ones[:], c0[:], None, ALU.mult)
                nc.sync.dma_start(out=cm2[1:2, :], in_=c0_row[:])

            # ---------------- per-chunk building blocks ----------------
            pending_xb = [x8c[c] for c in range(NCHUNK)]
            hstate = {}

            def emit_L1(c, s_order=(0, 1, 2, 3)):
                # s_order (0,2,1,3) for post-update cycles: the twist pairs
                # produce xb slabs {p, p+4}, so K-groups are consumed in the
                # order the casts retire; cycle 0's x8 DMAs arrive in slab
                # order, so the natural order is kept there.
                xb = pending_xb[c]
                pending_xb[c] = None
                h1 = work.tile([128, 8, CHUNK], f8, tag="h1")
                for fo in range(8):
                    ps1 = psm.tile([128, CHUNK], f32, tag="mm")
                    for si, s in enumerate(s_order):
                        nc.tensor.matmul(
                            ps1[:], w1f[:, fo, 2 * s:2 * s + 2, :],
                            xb[:, 2 * s:2 * s + 2, :],
                            start=(si == 0), stop=(si == 3), perf_mode=DR,
                        )
                    nc.scalar.activation(
                        h1[:, fo, :], ps1[:], AF.Gelu, bias=b1[:, fo:fo + 1])
                hstate[c] = h1

            def emit_L2(c):
                h1 = hstate[c]
                h2 = work.tile([128, 4, CHUNK], f8, tag="h2")
                for fo in range(4):
                    ps2 = psm.tile([128, CHUNK], f32, tag="mm")
                    for s in range(4):
                        nc.tensor.matmul(
                            ps2[:], w2f[:, fo, 2 * s:2 * s + 2, :],
                            h1[:, 2 * s:2 * s + 2, :],
                            start=(s == 0), stop=(s == 3), perf_mode=DR,
                        )
                    nc.scalar.activation(
                        h2[:, fo, :], ps2[:], AF.Gelu, bias=b2[:, fo:fo + 1])
                hstate[c] = h2

            def emit_L34(c):
                h2 = hstate.pop(c)
                h3 = work.tile([128, 2, CHUNK], f8, tag="h3")
                for fo in range(2):
                    ps3 = psm.tile([128, CHUNK], f32, tag="mm")
                    for s in range(2):
                        nc.tensor.matmul(
                            ps3[:], w3f[:, fo, 2 * s:2 * s + 2, :],
                            h2[:, 2 * s:2 * s + 2, :],
                            start=(s == 0), stop=(s == 1), perf_mode=DR,
                        )
                    nc.scalar.activation(
                        h3[:, fo, :], ps3[:], AF.Gelu, bias=b3[:, fo:fo + 1])
                # L4: M=1 forbids the DoubleRow ldweights layout -> 2 plain
                # fp8 matmuls (ISA check s3_lw_dual_fp8_restrictions)
                ps4 = psl4.tile([1, CHUNK], f32, tag="l4")
                for s in range(2):
                    nc.tensor.matmul(
                        ps4[:], w4f[:, s, :], h3[:, s, :],
                        start=(s == 0), stop=(s == 1),
                    )
                # th = tanh((z4 + b4)/2); sigmoid folded into the coupling
                nc.scalar.activation(
                    th_t[c][0:1, :], ps4[:], AF.Tanh, bias=b4h, scale=0.5)

            def mlp_chunk(c, hooks=()):
                """coupling-net MLP on chunk c of `out`.

                hooks: up to 3 closures emitted after L1/L2/L3+L4 — used to
                slot the previous chunks' coupling-broadcast + twist updates
                (and the tiny serial gc-net chain at cycle 0) into the queues
                at points where their ACT/DVE dependencies have had time to
                finish.
                """
                hooks = list(hooks) + [None] * 3
                emit_L1(c, s_order=(0, 2, 1, 3))
                if hooks[0]:
                    hooks[0]()
                emit_L2(c)
                if hooks[1]:
                    hooks[1]()
                emit_L34(c)
                if hooks[2]:
                    hooks[2]()

            def update_chunk(c, last, next_conv=False, k1=False):
                """coupling broadcast + twist update (in place) on chunk c;
                one consolidated DMA out if last.

                k1: two accumulating K=1 matmuls instead of the K=2 form —
                used for the first cycle-0 updates, which would otherwise
                wait on the cm2 row-1 SBUF->SBUF DMA (~2us latency)."""
                cb = pscb.tile([128, CHUNK], f32, tag="cb")
                if k1:
                    nc.tensor.matmul(
                        cb[:], cm2[0:1, :], th_t[c][0:1, :],
                        start=True, stop=False)
                    nc.tensor.matmul(
                        cb[:], c0_row[:], ones_row[:], start=False, stop=True)
                else:
                    # cb[p, j] = cmul'*th[j] + c0'*1  via one K=2 matmul
                    nc.tensor.matmul(
                        cb[:], cm2[:, :], th_t[c][:, :], start=True, stop=True)
                # one bf16 SBUF copy so the twist tensor_tensor ops all run
                # in the DVE 2x_1p fast mode (PSUM/f32 operands disable it)
                cbb = work.tile([128, CHUNK], bf16, tag="cbb")
                nc.vector.tensor_copy(cbb[:], cb[:])
                xb_next = None
                for p in range(4):
                    t, u = p, p + 4
                    tmpa = work.tile([128, CHUNK], bf16, tag="twa")
                    tmpb = work.tile([128, CHUNK], bf16, tag="twb")
                    if next_conv and xb_next is None:
                        xb_next = xbp.tile([128, 8, CHUNK], f8, tag="xb")
                    nc.vector.tensor_mul(tmpa[:], out_bf(u, c), cbb[:])
                    nc.vector.tensor_mul(tmpb[:], out_bf(t, c), cbb[:])
                    if TWIST_SIGN[t] > 0:
                        nc.vector.tensor_add(out_bf(t, c), out_bf(t, c), tmpa[:])
                    else:
                        nc.vector.tensor_sub(out_bf(t, c), out_bf(t, c), tmpa[:])
                    if next_conv:
                        # fp8 conversion per slab, immediately after its add:
                        # the next chunk's L1 K-groups unblock ~1us earlier
                        nc.vector.tensor_copy(xb_next[:, t, :], out_bf(t, c))
                    if TWIST_SIGN[u] > 0:
                        nc.vector.tensor_add(out_bf(u, c), out_bf(u, c), tmpb[:])
                    else:
                        nc.vector.tensor_sub(out_bf(u, c), out_bf(u, c), tmpb[:])
                    if next_conv:
                        nc.vector.tensor_copy(xb_next[:, u, :], out_bf(u, c))
                    if last and p % 2 == 1:
                        # pairs 0,1 finalize slabs {0,1,4,5}; pairs 2,3 the
                        # rest: ship each contiguous quarter as soon as its
                        # twist adds retire, one per HWDGE ring, so the
                        # transfers overlap the remaining update ops
                        lo = p - 1
                        nc.scalar.dma_start(
                            out=out_d[:, c, lo:lo + 2, :],
                            in_=xc[c][:, lo:lo + 2, :])
                        nc.sync.dma_start(
                            out=out_d[:, c, lo + 4:lo + 6, :],
                            in_=xc[c][:, lo + 4:lo + 6, :])
                if next_conv and not last:
                    pending_xb[c] = xb_next

            # ---------------- main cycles ----------------
            # Cycle 0, chunks 0+1 layer-paired: the in-order PE queue would
            # otherwise stall L1(1) behind L2(0)'s wait for chunk-0's trailing
            # GELU at the very start (ACT lags the first chunk's matmuls).
            # Reductions are emitted eagerly; the collective is issued as soon
            # as chunk 3's x lands; the serial gc-net stages ride mlp(2)'s
            # hooks (gmean arrives ~30us, well before); cycle-0 updates then
            # ride mlp(3)'s hooks so cycle-1 L1s start with zero boundary
            # stall.  From there every mlp hook emits the oldest pending
            # update (lag-2 at the boundary, catching back to lag-1 in cycle
            # 1 so the tail stays one update deep).
            load_x8p(0, nc.sync)
            load_x8p(1, nc.sync)
            load_x8p(2, nc.sync)
            load_x8p(3, nc.sync)
            emit_L1(0)
            reduce_chunk(0)
            emit_L2(0)
            emit_L34(0)
            emit_L1(1)
            reduce_chunk(1)
            reduce_chunk(2)
            reduce_chunk(3)
            reduce_partner(0)
            reduce_partner(1)
            reduce_partner(2)
            reduce_partner(3)
            finish_gsum()
            early[0] = 3
            load_gw()
            emit_L2(1)
            emit_L1(2)
            load_xc(0, nc.scalar)
            load_xc(1, nc.scalar)
            emit_L34(1)
            emit_L2(2)
            gc_stage1()
            emit_L1(3)
            gc_stage2()
            emit_L34(2)
            gc_stage3()

            def upd(c, last=False, k1=False):
                return lambda: update_chunk(
                    c, last, next_conv=not last, k1=k1)

            load_xc(2, nc.scalar)
            load_xc(3, nc.scalar)
            upd(0, k1=True)()
            emit_L2(3)
            upd(1, k1=True)()
            emit_L34(3)
            # cycle 1: catch from lag-2 back to lag-1
            mlp_chunk(0, hooks=(upd(2),))
            mlp_chunk(1, hooks=(upd(3), None, upd(0)))
            mlp_chunk(2, hooks=(upd(1),))
            mlp_chunk(3, hooks=(upd(2),))
            # cycle 2 (last): updates write the final output + DMA out.
            # The last two chunks are layer-paired so the end-of-stream
            # GELU-latency waits are filled with the other chunk's matmuls.
            mlp_chunk(0, hooks=(upd(3),))
            mlp_chunk(1, hooks=(upd(0, last=True),))
            emit_L1(2, s_order=(0, 2, 1, 3))
            upd(1, last=True)()
            emit_L2(2)
            emit_L1(3, s_order=(0, 2, 1, 3))
            emit_L34(2)
            upd(2, last=True)()
            emit_L2(3)
            emit_L34(3)
            update_chunk(3, last=True, next_conv=False)

    nc.compile()
    return nc


def _get_graph():
    if "nc" not in _CACHE:
        _CACHE["nc"] = _build_graph()
    return _CACHE["nc"]


def _pack_consts(inputs):
    cst = np.zeros((128, 25), np.float32)
    cst[:, 0:8] = np.asarray(inputs["cn_b1"], np.float32).reshape(8, 128).T
    cst[:, 8:12] = np.asarray(inputs["cn_b2"], np.float32).reshape(4, 128).T
    cst[:, 12:14] = np.asarray(inputs["cn_b3"], np.float32).reshape(2, 128).T
    cst[:, 14:18] = np.asarray(inputs["gc_b1"], np.float32).reshape(4, 128).T
    cst[:, 18:20] = np.asarray(inputs["gc_b2"], np.float32).reshape(2, 128).T
    cst[0, 20] = 0.5 * np.asarray(inputs["cn_b4"], np.float32).reshape(())
    cst[0, 21] = 0.5 * np.asarray(inputs["gc_b3"], np.float32).reshape(())
    arv = np.asarray(inputs["adaptive_range"], np.float32).reshape(())
    cst[0, 22] = arv
    cst[0, 23] = 0.7 * arv
    cst[0, 24] = 0.3 * arv
    return cst


def _make_in_maps(inputs):
    import ml_dtypes

    bf = ml_dtypes.bfloat16
    f8 = ml_dtypes.float8_e4m3
    x = np.ascontiguousarray(inputs["x"], dtype=np.float32)
    # [core, p, chunk, slab, tok] — exact SBUF tile layout, so every DMA is
    # partition-contiguous: x[b, s, d] with token j = chunk*512+tok,
    # feature f = slab*128+p
    xs = (x.reshape(NCORES, NCHUNK, CHUNK, 8, 128)
          .transpose(0, 4, 1, 3, 2))  # [8, 128, 4, 8, 512]
    xs = np.ascontiguousarray(xs)

    def pack_w(w, nfo):
        # [s*128+p, fo*128+j] -> [p, fo, s, j]
        kin = w.shape[0]
        return np.ascontiguousarray(
            np.asarray(w).reshape(kin // 128, 128, nfo, 128)
            .transpose(1, 2, 0, 3), dtype=f8)

    def pack_gw(w):
        # [k*128+p, m] -> [p, k, m]
        kin, m = w.shape
        return np.ascontiguousarray(
            np.asarray(w).reshape(kin // 128, 128, m).transpose(1, 0, 2),
            dtype=bf)

    shared = {
        "cn_w1": pack_w(inputs["cn_w1"], 8),
        "cn_w2": pack_w(inputs["cn_w2"], 4),
        "cn_w3": pack_w(inputs["cn_w3"], 2),
        "cn_w4": np.ascontiguousarray(
            np.asarray(inputs["cn_w4"]).reshape(2, 128).T.reshape(128, 2, 1),
            dtype=f8),
        "gc_w1": pack_gw(np.asarray(inputs["gc_w1"])),
        "gc_w2": pack_gw(np.asarray(inputs["gc_w2"])),
        "gc_w3": pack_gw(np.asarray(inputs["gc_w3"]).reshape(256, 1)),
        "consts": _pack_consts(inputs),
    }
    xs8 = xs.astype(f8)
    in_maps = []
    for c in range(NCORES):
        m = dict(shared)
        m["x"] = xs[c].astype(bf)
        m["x8"] = xs8[c]
        m["x8p"] = xs8[c ^ 1]  # partner = other half of the same batch row
        in_maps.append(m)
    return in_maps


def _run(inputs, trace=False):
    from concourse.bass_utils import run_bass_kernel_spmd

    nc = _get_graph()
    in_maps = _make_in_maps(inputs)
    res = run_bass_kernel_spmd(
        nc, in_maps, core_ids=list(range(NCORES)), trace=trace
    )
    # out[p, chunk, slab, tok] -> [token = chunk*512+tok, feat = slab*128+p]
    outs = np.stack(
        [np.asarray(res.results[c]["out"]).astype(np.float32)
         .transpose(1, 3, 2, 0).reshape(TOK, DIM)
         for c in range(NCORES)], axis=0
    )  # [8, 2048, 1024]
    full = outs.reshape(B, S, DIM).astype(np.float32)
    return full, res


def kernel(**inputs) -> np.ndarray:
    out, _ = _run(inputs, trace=False)
    return out
